# revision 19
# baseline (speedup 1.0000x reference)
"""Trainium2 Bass kernel for nn_AEGConv2d (8 NeuronCores, SPMD).

Problem: out = sigmoid(aeg(x, weight)) * (conv2d(x, conv_w) + conv_b)
  x: (4, 32, 64, 64) f32, weight/conv_w: (64, 32, 3, 3), conv_b: (64,)
  stride=1, padding=1.

The AEG recurrence  res <- where(mask_k, (res+x_k)*y_k, (res+y_k)*x_k)
is affine in res with b_k = x_k*y_k in both branches and multiplier
a_k = y_k (mask true) or x_k (mask false).  mask(k, i, j) = (i+j+k)%2==0
is a checkerboard, so for a pixel of parity s=(i+j)%2 the unrolled sum
    res = sum_k x_k*y_k * prod_{j>k} a_j
factors into a per-pixel product A_k = x_k * prod_{j>k, j%2!=s} x_j and a
per-(cout,cin) product B_k = y_k * prod_{j>k, j%2==s} y_j, making the whole
AEG conv a 288-deep matmul per parity class -- same shape as the dense conv.

Sharding: 8 cores = 4 images x 2 row-halves (rows 0:32 / 32:64). Each core
computes all 64 Cout for its half image. No collectives; host gathers.

Layout: the host packs the padded input slab into bf16 checkerboard parity
planes plane_q[cin, r, ch] = xp[cin, r, 2*ch + (q+r)%2] (plus one-element-
shifted copies of each plane) so that every tap view -- both the per-parity
elementwise views and the matmul rhs grids -- is a unit-minor-stride,
4B-aligned AP of a single 32-partition SBUF tensor.  bf16 gives the DVE 2x
mode (720ns vs 1360ns per 1024-elem op) and the PE 1 cycle/row matmuls.
"""

import numpy as np
import ml_dtypes

import concourse.bacc as bacc
import concourse.bass as bass
import concourse.mybir as mybir
import concourse.tile as tile
from concourse.bass_utils import run_bass_kernel_spmd

F32 = mybir.dt.float32
BF16 = mybir.dt.bfloat16

N, CIN, H, W = 4, 32, 64, 64
COUT, KK = 64, 3
PAD = 1
OH, OW = 32, 64          # per-core output rows x cols
ROWS, COLS = 34, 66      # per-core padded slab
PLP = 34                 # plane row pitch (even for alignment)
PLSZ = PLP * ROWS        # 1156 elements per plane per cin
N_CORES = 8

# suffix level needed by A_k at parity s (0 => raw tap, direct matmul)
SIGMA = {
    0: {0: 4, 1: 3, 2: 3, 3: 2, 4: 2, 5: 1, 6: 1, 7: 0, 8: 0},
    1: {0: 4, 1: 4, 2: 3, 3: 3, 4: 2, 5: 2, 6: 1, 7: 1, 8: 0},
}
CHAIN_TAPS = {0: [7, 5, 3, 1], 1: [8, 6, 4, 2]}
# kj -> raw tap handled by a K=32 matmul on the raw plane view
RAW = {0: {1: 7, 2: 8}, 1: {2: 8}}
# chain level L (2,3,4) coincides with A_k for these taps:
#   s=0: A_5=O2, A_3=O3, A_1=O4 ; s=1: A_6=E2, A_4=E3, A_2=E4
CHAIN_IS_A = {0: {2: 5, 3: 3, 4: 1}, 1: {2: 6, 3: 4, 4: 2}}
# A-tile row order per (s, kj): chain-value tap first (row 0), then the
# mul taps, then raw (if any; its rows stay unwritten and are handled by
# a braw K=32 matmul).  Host packs the B columns in the same order.
ROW_ORDER = {
    (0, 0): [3, 0, 6], (0, 1): [1, 4, 7], (0, 2): [5, 2, 8],
    (1, 0): [6, 0, 3], (1, 1): [4, 1, 7], (1, 2): [2, 5, 8],
}

_last_results = None  # stash for test.py (exec_time_ns etc.)


def _fview(base_ap, off, dims):
    """View with the same partition dim as base_ap but custom free dims."""
    return bass.AP(
        tensor=base_ap.tensor,
        offset=base_ap.offset + off,
        ap=[base_ap.ap[0]] + dims,
    )


def _plane_off(k, s, t):
    """Aligned offset of tap k, parity s, grid t inside the XP tensor.

    XP free layout: [plane0 | plane1 | plane0shift | plane1shift], each PLSZ.
    Element (r', ch) of plane q is at q*PLSZ + r'*PLP + ch; the shifted copies
    hold plane[q][..., ch+1] so an odd offset o in plane q equals the even
    offset o-1 in plane q+2.
    """
    ki, kj = divmod(k, 3)
    q = (s + ki + kj) % 2
    m = ((s ^ t) + kj) // 2
    off = q * PLSZ + (t + ki) * PLP + m
    if off % 2 == 1:
        off = (2 + q) * PLSZ + (off - q * PLSZ) - 1
    return off


def build_nc():
    nc = bacc.Bacc(None, target_bir_lowering=False)
    xp_d = nc.declare_dram_parameter("xp", [CIN, 4 * PLSZ], BF16, isOutput=False)
    xa_d = nc.declare_dram_parameter("xa", [96, 2 * PLSZ], BF16, isOutput=False)
    xb_d = nc.declare_dram_parameter("xb", [96, 2 * PLSZ], BF16, isOutput=False)
    wm0_d = nc.declare_dram_parameter("wm0", [96, 3, 128], BF16, isOutput=False)
    wm1_d = nc.declare_dram_parameter("wm1", [96, 3, 128], BF16, isOutput=False)
    b0_d = nc.declare_dram_parameter("b0", [96, 3, COUT], BF16, isOutput=False)
    b1_d = nc.declare_dram_parameter("b1", [96, 3, COUT], BF16, isOutput=False)
    bias_d = nc.declare_dram_parameter("bias", [COUT, 1], F32, isOutput=False)
    out_d = nc.declare_dram_parameter("out", [COUT, OH * OW], BF16, isOutput=True)

    with tile.TileContext(nc) as tc:
        with (
            tc.tile_pool(name="big", bufs=1) as big,
            tc.tile_pool(name="sig", bufs=4) as sigp,
            tc.tile_pool(name="psum", bufs=1, space="PSUM") as pp,
        ):
            # --- load inputs: one DMA per plane copy (4 queues in parallel) ---
            XP = big.tile([CIN, 4 * PLSZ], BF16)
            xpb = xp_d[:, :]
            dmae = [nc.sync, nc.scalar]
            HP = PLSZ // 2
            for i in range(8):
                dmae[i % 2].dma_start(
                    out=XP[:, HP * i : HP * (i + 1)],
                    in_=bass.AP(
                        tensor=xpb.tensor,
                        offset=HP * i,
                        ap=[[4 * PLSZ, CIN], [1, HP]],
                    ),
                )
            XAB = {}
            for idx, (c, dram) in enumerate(((0, xa_d), (1, xb_d))):
                t = big.tile([96, 2 * PLSZ], BF16, tag=f"xab{c}")
                for j in range(2):
                    dmae[(idx + j) % 2].dma_start(
                        out=t[:, PLSZ * j : PLSZ * (j + 1)],
                        in_=bass.AP(
                            tensor=dram[:, :].tensor,
                            offset=PLSZ * j,
                            ap=[[2 * PLSZ, 96], [1, PLSZ]],
                        ),
                    )
                XAB[c] = t
            wts = {}
            for idx, (name, dram, shape) in enumerate((
                    ("wm0", wm0_d, [96, 3, 128]),
                    ("wm1", wm1_d, [96, 3, 128]),
                    ("b0", b0_d, [96, 3, COUT]),
                    ("b1", b1_d, [96, 3, COUT]))):
                t = big.tile(shape, BF16, tag=name)
                dmae[idx % 2].dma_start(out=t[:, :, :], in_=dram[:, :, :])
                wts[name] = t
            bias_t = big.tile([COUT, 1], F32)
            nc.sync.dma_start(out=bias_t[:, :], in_=bias_d[:, :])

            out_sb = big.tile([COUT, OH * OW], BF16)


            # Touch ops: absorb DMA-completion waits into DVE program order so
            # every TensorTensor needs at most one embedded sync wait.
            scratch = big.tile([1, 8], F32)
            touch_srcs = [XP[0:1, HP * i : HP * i + 1] for i in range(8)]
            touch_srcs.append(bias_t[0:1, 0:1])
            for tt in touch_srcs:
                nc.vector.tensor_copy(scratch[0:1, 0:1], tt)

            xp_all = XP[:, :]

            def tapview(k, s):
                """(32, 2,16,32) aligned unit-stride view: tap k, all parity-s."""
                b0 = _plane_off(k, s, 0)
                b1 = _plane_off(k, s, 1)
                return _fview(xp_all, b0, [[b1 - b0, 2], [2 * PLP, 16], [1, 32]])

            def convgrid(kj, s, t):
                """(96, 16,32) K=96 rhs: kernel-column kj, grid t, parity s.
                Partition group r of XA/XB holds the r-row-shifted plane
                (c+r)%2 with c = (s+kj)%2, plus its col-shifted copy."""
                c = (s + kj) % 2
                m = ((s ^ t) + kj) // 2
                off = t * PLP + m
                if off % 2 == 1:
                    off = PLSZ + off - 1
                return _fview(XAB[c][:, :], off, [[2 * PLP, 16], [1, 32]])

            # --- chains + A tensors (all DVE; bf16 2x mode) ---
            # Chain level L IS A_{CHAIN_IS_A[s][L]}, written straight into its
            # A-tile's row 0 (base partition 0, so the next chain step can
            # read it as an input).  Only 9 extra A-muls remain.
            A = {}
            rowof = {}
            for s in (0, 1):
                for kj in range(3):
                    at = big.tile([96, 2, 16, 32], BF16, tag=f"A{s}{kj}")
                    A[(s, kj)] = at
                    for ri, k in enumerate(ROW_ORDER[(s, kj)]):
                        rowof[(s, k)] = (at, ri)
            for s in (1, 0):
                c = CHAIN_TAPS[s]
                chain = {1: tapview(c[0], s)}
                for lvl in (2, 3, 4):
                    at, ri = rowof[(s, CHAIN_IS_A[s][lvl])]
                    assert ri == 0
                    nc.vector.tensor_mul(
                        at[0:32, :, :, :], tapview(c[lvl - 1], s), chain[lvl - 1]
                    )
                    chain[lvl] = at[0:32, :, :, :]
                for k in range(9):
                    lvl = SIGMA[s][k]
                    if lvl == 0 or k == CHAIN_IS_A[s].get(lvl):
                        continue  # raw tap, or already produced by the chain
                    at, ri = rowof[(s, k)]
                    nc.vector.tensor_mul(
                        at[32 * ri : 32 * ri + 32, :, :, :],
                        tapview(k, s),
                        chain[lvl],
                    )

            # --- matmuls: all conv groups first (inputs ready early), then
            # aeg groups (gated on A-tiles), then epilogues.  This keeps the
            # PE program order free of stalls: a later quadrant's conv never
            # sits behind an earlier quadrant's A-gated aeg matmul.
            bsn = {0: "b0", 1: "b1"}
            psq = {}
            for s in (0, 1):
                for t in (0, 1):
                    ps = pp.tile([128, 16, 32], F32, tag=f"ps{s}{t}")
                    psq[(s, t)] = ps
                    wm = wts[f"wm{s}"]
                    for kj in range(3):
                        nc.tensor.matmul(
                            ps[:, :, :],
                            wm[:, kj, :],
                            convgrid(kj, s, t),
                            start=(kj == 0),
                            stop=False,
                            skip_group_check=True,
                        )
            for s in (0, 1):
                for t in (0, 1):
                    ps = psq[(s, t)]
                    bt = wts[bsn[s]]
                    mms = []
                    for kj in range(3):
                        if ROW_ORDER[(s, kj)][2] == RAW[s].get(kj):
                            mms.append((bt[0:64, kj, :], A[(s, kj)][0:64, t, :, :]))
                        else:
                            mms.append((bt[:, kj, :], A[(s, kj)][:, t, :, :]))
                    for i, (lh, rh) in enumerate(mms):
                        nc.tensor.matmul(
                            ps[0:64, :, :],
                            lh,
                            rh,
                            start=False,
                            stop=(i == len(mms) - 1),
                            skip_group_check=True,
                        )
            for s in (0, 1):
                for t in (0, 1):
                    ps = psq[(s, t)]
                    sig = sigp.tile([64, 16, 32], F32)
                    nc.scalar.activation(
                        sig[:, :, :], ps[0:64, :, :],
                        mybir.ActivationFunctionType.Sigmoid,
                    )
                    # absorb the ACT-completion wait so the STT below only
                    # waits on PSUM (one embedded sync wait max)
                    nc.vector.tensor_copy(scratch[0:1, 0:1], sig[0:1, 0:1, 0:1])
                    # out = (conv + bias) * sigmoid(aeg), scattered to parity pixels
                    ov = _fview(out_sb[:, :], 64 * t + (s ^ t), [[128, 16], [2, 32]])
                    nc.vector.scalar_tensor_tensor(
                        out=ov,
                        in0=ps[64:128, :, :],
                        scalar=bias_t[:, 0:1],
                        in1=sig[:, :, :],
                        op0=mybir.AluOpType.add,
                        op1=mybir.AluOpType.mult,
                    )

            for i in range(8):
                dmae[i % 2].dma_start(
                    out=out_d[8 * i : 8 * i + 8, :],
                    in_=out_sb[8 * i : 8 * i + 8, :],
                )
    nc.finalize()
    return nc


def _host_prep(x, weight, conv_w, conv_b):
    """Shard + pack per-core inputs (bf16 parity planes + weight products)."""
    bf16 = ml_dtypes.bfloat16
    xp = np.pad(np.ascontiguousarray(x, np.float32),
                ((0, 0), (0, 0), (PAD, PAD), (PAD, PAD)))
    kflat = weight.reshape(COUT, CIN, 9).transpose(2, 0, 1)  # (9, cout, cin)
    B = np.zeros((2, 9, COUT, CIN), np.float32)
    for s in (0, 1):
        suf = np.ones((COUT, CIN), np.float32)
        for k in range(8, -1, -1):
            B[s, k] = kflat[k] * suf
            if k % 2 == s:
                suf = suf * kflat[k]
    # merged conv lhsT per parity: (96, 3, 128): rows (ki,cin), cols
    # [B_raw (aeg) | Wc (conv)]; B_raw nonzero only where SIGMA[s][k]==0
    wc_k = conv_w.reshape(COUT, CIN, 9)  # (cout, cin, k)
    wm_p = np.zeros((2, 96, 3, 128), np.float32)
    for s in (0, 1):
        for ki in range(3):
            for kj in range(3):
                k = ki * 3 + kj
                blk = slice(32 * ki, 32 * ki + 32)
                wm_p[s, blk, kj, 64:128] = wc_k[:, :, k].T
                if SIGMA[s][k] == 0:
                    wm_p[s, blk, kj, 0:64] = B[s, k].T
    wm_p = wm_p.astype(bf16)
    ROW_ORDER = {
        (0, 0): [3, 0, 6], (0, 1): [1, 4, 7], (0, 2): [5, 2, 8],
        (1, 0): [6, 0, 3], (1, 1): [4, 1, 7], (1, 2): [2, 5, 8],
    }
    b_p = np.zeros((2, 96, 3, COUT), np.float32)
    for s in (0, 1):
        for kj in range(3):
            for ri, k in enumerate(ROW_ORDER[(s, kj)]):
                b_p[s, 32 * ri : 32 * ri + 32, kj] = B[s, k].T
    b_p = b_p.astype(bf16)
    bias_p = np.ascontiguousarray(conv_b.reshape(COUT, 1), np.float32)

    in_maps = []
    for core in range(N_CORES):
        n, h = divmod(core, 2)
        slab = xp[n, :, 32 * h : 32 * h + ROWS, :]  # (32, 34, 66) f32
        planes = np.zeros((4, CIN, ROWS, PLP), np.float32)
        for q in (0, 1):
            for r in range(ROWS):
                b = (q + r) % 2
                cols = slab[:, r, b::2]  # 33 columns
                planes[q, :, r, : cols.shape[1]] = cols
        planes[2, :, :, :PLP - 1] = planes[0, :, :, 1:]
        planes[3, :, :, :PLP - 1] = planes[1, :, :, 1:]
        xp_core = np.ascontiguousarray(
            planes.transpose(1, 0, 2, 3).reshape(CIN, 4 * PLSZ)
        ).astype(bf16)
        # partition-stacked row-shifted plane sets for the K=96 conv chunks:
        # xa group r holds plane (r%2) shifted down r rows (+ col-shift copy),
        # xb group r holds plane ((r+1)%2) likewise.
        xab = np.zeros((2, 3, CIN, 2, ROWS, PLP), np.float32)
        for c in (0, 1):
            for r in range(3):
                q = (c + r) % 2
                xab[c, r, :, 0, : ROWS - r] = planes[q, :, r:]
                xab[c, r, :, 1, : ROWS - r] = planes[2 + q, :, r:]
        xa_core = np.ascontiguousarray(xab[0].reshape(96, 2 * PLSZ)).astype(bf16)
        xb_core = np.ascontiguousarray(xab[1].reshape(96, 2 * PLSZ)).astype(bf16)
        in_maps.append({
            "xp": xp_core,
            "xa": xa_core,
            "xb": xb_core,
            "wm0": wm_p[0],
            "wm1": wm_p[1],
            "b0": b_p[0],
            "b1": b_p[1],
            "bias": bias_p,
        })
    return in_maps


_nc_cache = None


def kernel(x, weight, conv_w, conv_b, trace=False):
    global _nc_cache, _last_results
    x = np.asarray(x, np.float32)
    weight = np.asarray(weight, np.float32)
    conv_w = np.asarray(conv_w, np.float32)
    conv_b = np.asarray(conv_b, np.float32)

    if _nc_cache is None:
        _nc_cache = build_nc()
    nc = _nc_cache
    in_maps = _host_prep(x, weight, conv_w, conv_b)
    res = run_bass_kernel_spmd(nc, in_maps, core_ids=list(range(N_CORES)), trace=trace)
    _last_results = res

    out = np.empty((N, COUT, H, W), np.float32)
    for core in range(N_CORES):
        n, h = divmod(core, 2)
        out[n, :, 32 * h : 32 * h + 32, :] = (
            res.results[core]["out"].astype(np.float32).reshape(COUT, OH, OW)
        )
    return out


# revision 20
# speedup vs baseline: 1.0485x; 1.0485x over previous
"""Trainium2 Bass kernel for nn_AEGConv2d (8 NeuronCores, SPMD).

Problem: out = sigmoid(aeg(x, weight)) * (conv2d(x, conv_w) + conv_b)
  x: (4, 32, 64, 64) f32, weight/conv_w: (64, 32, 3, 3), conv_b: (64,)
  stride=1, padding=1.

The AEG recurrence  res <- where(mask_k, (res+x_k)*y_k, (res+y_k)*x_k)
is affine in res with b_k = x_k*y_k in both branches and multiplier
a_k = y_k (mask true) or x_k (mask false).  mask(k, i, j) = (i+j+k)%2==0
is a checkerboard, so for a pixel of parity s=(i+j)%2 the unrolled sum
    res = sum_k x_k*y_k * prod_{j>k} a_j
factors into a per-pixel product A_k = x_k * prod_{j>k, j%2!=s} x_j and a
per-(cout,cin) product B_k = y_k * prod_{j>k, j%2==s} y_j, making the whole
AEG conv a 288-deep matmul per parity class -- same shape as the dense conv.

Sharding: 8 cores = 4 images x 2 row-halves (rows 0:32 / 32:64). Each core
computes all 64 Cout for its half image. No collectives; host gathers.

Layout: the host packs the padded input slab into bf16 checkerboard parity
planes plane_q[cin, r, ch] = xp[cin, r, 2*ch + (q+r)%2] (plus one-element-
shifted copies of each plane) so that every tap view -- both the per-parity
elementwise views and the matmul rhs grids -- is a unit-minor-stride,
4B-aligned AP of a single 32-partition SBUF tensor.  bf16 gives the DVE 2x
mode (720ns vs 1360ns per 1024-elem op) and the PE 1 cycle/row matmuls.
"""

import numpy as np
import ml_dtypes

import concourse.bacc as bacc
import concourse.bass as bass
import concourse.mybir as mybir
import concourse.tile as tile
from concourse.bass_utils import run_bass_kernel_spmd

F32 = mybir.dt.float32
BF16 = mybir.dt.bfloat16

N, CIN, H, W = 4, 32, 64, 64
COUT, KK = 64, 3
PAD = 1
OH, OW = 32, 64          # per-core output rows x cols
ROWS, COLS = 34, 66      # per-core padded slab
PLP = 34                 # plane row pitch (even for alignment)
PLSZ = PLP * ROWS        # 1156 elements per plane per cin
N_CORES = 8

# suffix level needed by A_k at parity s (0 => raw tap, direct matmul)
SIGMA = {
    0: {0: 4, 1: 3, 2: 3, 3: 2, 4: 2, 5: 1, 6: 1, 7: 0, 8: 0},
    1: {0: 4, 1: 4, 2: 3, 3: 3, 4: 2, 5: 2, 6: 1, 7: 1, 8: 0},
}
CHAIN_TAPS = {0: [7, 5, 3, 1], 1: [8, 6, 4, 2]}
# kj -> raw tap handled by a K=32 matmul on the raw plane view
RAW = {0: {1: 7, 2: 8}, 1: {2: 8}}
# chain level L (2,3,4) coincides with A_k for these taps:
#   s=0: A_5=O2, A_3=O3, A_1=O4 ; s=1: A_6=E2, A_4=E3, A_2=E4
CHAIN_IS_A = {0: {2: 5, 3: 3, 4: 1}, 1: {2: 6, 3: 4, 4: 2}}
# A-tile row order per (s, kj): chain-value tap first (row 0), then the
# mul taps, then raw (if any; its rows stay unwritten and are handled by
# a braw K=32 matmul).  Host packs the B columns in the same order.
ROW_ORDER = {
    (0, 0): [3, 0, 6], (0, 1): [1, 4, 7], (0, 2): [5, 2, 8],
    (1, 0): [6, 0, 3], (1, 1): [4, 1, 7], (1, 2): [2, 5, 8],
}

_last_results = None  # stash for test.py (exec_time_ns etc.)


def _fview(base_ap, off, dims):
    """View with the same partition dim as base_ap but custom free dims."""
    return bass.AP(
        tensor=base_ap.tensor,
        offset=base_ap.offset + off,
        ap=[base_ap.ap[0]] + dims,
    )


def _plane_off(k, s, t):
    """Aligned offset of tap k, parity s, grid t inside the XP tensor.

    XP free layout: [plane0 | plane1 | plane0shift | plane1shift], each PLSZ.
    Element (r', ch) of plane q is at q*PLSZ + r'*PLP + ch; the shifted copies
    hold plane[q][..., ch+1] so an odd offset o in plane q equals the even
    offset o-1 in plane q+2.
    """
    ki, kj = divmod(k, 3)
    q = (s + ki + kj) % 2
    m = ((s ^ t) + kj) // 2
    off = q * PLSZ + (t + ki) * PLP + m
    if off % 2 == 1:
        off = (2 + q) * PLSZ + (off - q * PLSZ) - 1
    return off


def build_nc():
    nc = bacc.Bacc(None, target_bir_lowering=False)
    xp_d = nc.declare_dram_parameter("xp", [CIN, 4 * PLSZ], BF16, isOutput=False)
    xa_d = nc.declare_dram_parameter("xa", [96, 2 * PLSZ], BF16, isOutput=False)
    xb_d = nc.declare_dram_parameter("xb", [96, 2 * PLSZ], BF16, isOutput=False)
    wm0_d = nc.declare_dram_parameter("wm0", [96, 3, 128], BF16, isOutput=False)
    wm1_d = nc.declare_dram_parameter("wm1", [96, 3, 128], BF16, isOutput=False)
    b0_d = nc.declare_dram_parameter("b0", [96, 3, COUT], BF16, isOutput=False)
    b1_d = nc.declare_dram_parameter("b1", [96, 3, COUT], BF16, isOutput=False)
    bias_d = nc.declare_dram_parameter("bias", [COUT, 1], F32, isOutput=False)
    out_d = nc.declare_dram_parameter("out", [COUT, OH * OW], BF16, isOutput=True)

    with tile.TileContext(nc) as tc:
        with (
            tc.tile_pool(name="big", bufs=1) as big,
            tc.tile_pool(name="sig", bufs=4) as sigp,
            tc.tile_pool(name="psum", bufs=1, space="PSUM") as pp,
        ):
            # --- load inputs: one DMA per plane copy (4 queues in parallel) ---
            XP = big.tile([CIN, 4 * PLSZ], BF16)
            xpb = xp_d[:, :]
            dmae = [nc.sync, nc.scalar]
            HP = PLSZ // 2
            for i in range(8):
                dmae[i % 2].dma_start(
                    out=XP[:, HP * i : HP * (i + 1)],
                    in_=bass.AP(
                        tensor=xpb.tensor,
                        offset=HP * i,
                        ap=[[4 * PLSZ, CIN], [1, HP]],
                    ),
                )
            XAB = {}
            for idx, (c, dram) in enumerate(((0, xa_d), (1, xb_d))):
                t = big.tile([96, 2 * PLSZ], BF16, tag=f"xab{c}")
                for j in range(2):
                    dmae[(idx + j) % 2].dma_start(
                        out=t[:, PLSZ * j : PLSZ * (j + 1)],
                        in_=bass.AP(
                            tensor=dram[:, :].tensor,
                            offset=PLSZ * j,
                            ap=[[2 * PLSZ, 96], [1, PLSZ]],
                        ),
                    )
                XAB[c] = t
            wts = {}
            for idx, (name, dram, shape) in enumerate((
                    ("wm0", wm0_d, [96, 3, 128]),
                    ("wm1", wm1_d, [96, 3, 128]),
                    ("b0", b0_d, [96, 3, COUT]),
                    ("b1", b1_d, [96, 3, COUT]))):
                t = big.tile(shape, BF16, tag=name)
                dmae[idx % 2].dma_start(out=t[:, :, :], in_=dram[:, :, :])
                wts[name] = t
            bias_t = big.tile([COUT, 1], F32)
            nc.sync.dma_start(out=bias_t[:, :], in_=bias_d[:, :])

            out_sb = big.tile([COUT, OH * OW], BF16)


            # Touch ops: absorb DMA-completion waits into DVE program order so
            # every TensorTensor needs at most one embedded sync wait.
            scratch = big.tile([1, 8], F32)
            touch_srcs = [XP[0:1, HP * i : HP * i + 1] for i in range(8)]
            touch_srcs.append(bias_t[0:1, 0:1])
            for tt in touch_srcs:
                nc.vector.tensor_copy(scratch[0:1, 0:1], tt)

            xp_all = XP[:, :]

            def tapview(k, s):
                """(32, 2,16,32) aligned unit-stride view: tap k, all parity-s."""
                b0 = _plane_off(k, s, 0)
                b1 = _plane_off(k, s, 1)
                return _fview(xp_all, b0, [[b1 - b0, 2], [2 * PLP, 16], [1, 32]])

            def convgrid(kj, s, t):
                """(96, 16,32) K=96 rhs: kernel-column kj, grid t, parity s.
                Partition group r of XA/XB holds the r-row-shifted plane
                (c+r)%2 with c = (s+kj)%2, plus its col-shifted copy."""
                c = (s + kj) % 2
                m = ((s ^ t) + kj) // 2
                off = t * PLP + m
                if off % 2 == 1:
                    off = PLSZ + off - 1
                return _fview(XAB[c][:, :], off, [[2 * PLP, 16], [1, 32]])

            # --- chains + A tensors (all DVE; bf16 2x mode) ---
            # Chain level L IS A_{CHAIN_IS_A[s][L]}, written straight into its
            # A-tile's row 0 (base partition 0, so the next chain step can
            # read it as an input).  Only 9 extra A-muls remain.
            A = {}
            rowof = {}
            for s in (0, 1):
                for kj in range(3):
                    at = big.tile([96, 2, 16, 32], BF16, tag=f"A{s}{kj}")
                    A[(s, kj)] = at
                    for ri, k in enumerate(ROW_ORDER[(s, kj)]):
                        rowof[(s, k)] = (at, ri)
            for s in (1, 0):
                c = CHAIN_TAPS[s]
                chain = {1: tapview(c[0], s)}
                for lvl in (2, 3, 4):
                    at, ri = rowof[(s, CHAIN_IS_A[s][lvl])]
                    assert ri == 0
                    nc.vector.tensor_mul(
                        at[0:32, :, :, :], tapview(c[lvl - 1], s), chain[lvl - 1]
                    )
                    chain[lvl] = at[0:32, :, :, :]
                for k in range(9):
                    lvl = SIGMA[s][k]
                    if lvl == 0 or k == CHAIN_IS_A[s].get(lvl):
                        continue  # raw tap, or already produced by the chain
                    at, ri = rowof[(s, k)]
                    nc.vector.tensor_mul(
                        at[32 * ri : 32 * ri + 32, :, :, :],
                        tapview(k, s),
                        chain[lvl],
                    )

            # --- matmuls: all conv groups first (inputs ready early), then
            # aeg groups (gated on A-tiles), then epilogues.  This keeps the
            # PE program order free of stalls: a later quadrant's conv never
            # sits behind an earlier quadrant's A-gated aeg matmul.
            bsn = {0: "b0", 1: "b1"}
            psq = {}
            for s in (1, 0):
                for t in (0, 1):
                    ps = pp.tile([128, 16, 32], F32, tag=f"ps{s}{t}")
                    psq[(s, t)] = ps
                    wm = wts[f"wm{s}"]
                    for kj in range(3):
                        nc.tensor.matmul(
                            ps[:, :, :],
                            wm[:, kj, :],
                            convgrid(kj, s, t),
                            start=(kj == 0),
                            stop=False,
                            skip_group_check=True,
                        )
            for s in (1, 0):
                for t in (0, 1):
                    ps = psq[(s, t)]
                    bt = wts[bsn[s]]
                    mms = []
                    for kj in range(3):
                        if ROW_ORDER[(s, kj)][2] == RAW[s].get(kj):
                            mms.append((bt[0:64, kj, :], A[(s, kj)][0:64, t, :, :]))
                        else:
                            mms.append((bt[:, kj, :], A[(s, kj)][:, t, :, :]))
                    for i, (lh, rh) in enumerate(mms):
                        nc.tensor.matmul(
                            ps[0:64, :, :],
                            lh,
                            rh,
                            start=False,
                            stop=(i == len(mms) - 1),
                            skip_group_check=True,
                        )
            for s in (1, 0):
                for t in (0, 1):
                    ps = psq[(s, t)]
                    sig = sigp.tile([64, 16, 32], F32)
                    nc.scalar.activation(
                        sig[:, :, :], ps[0:64, :, :],
                        mybir.ActivationFunctionType.Sigmoid,
                    )
                    # absorb the ACT-completion wait so the STT below only
                    # waits on PSUM (one embedded sync wait max)
                    nc.vector.tensor_copy(scratch[0:1, 0:1], sig[0:1, 0:1, 0:1])
                    # out = (conv + bias) * sigmoid(aeg), scattered to parity pixels
                    ov = _fview(out_sb[:, :], 64 * t + (s ^ t), [[128, 16], [2, 32]])
                    nc.vector.scalar_tensor_tensor(
                        out=ov,
                        in0=ps[64:128, :, :],
                        scalar=bias_t[:, 0:1],
                        in1=sig[:, :, :],
                        op0=mybir.AluOpType.add,
                        op1=mybir.AluOpType.mult,
                    )

            for i in range(8):
                dmae[i % 2].dma_start(
                    out=out_d[8 * i : 8 * i + 8, :],
                    in_=out_sb[8 * i : 8 * i + 8, :],
                )
    nc.finalize()
    return nc


def _host_prep(x, weight, conv_w, conv_b):
    """Shard + pack per-core inputs (bf16 parity planes + weight products)."""
    bf16 = ml_dtypes.bfloat16
    xp = np.pad(np.ascontiguousarray(x, np.float32),
                ((0, 0), (0, 0), (PAD, PAD), (PAD, PAD)))
    kflat = weight.reshape(COUT, CIN, 9).transpose(2, 0, 1)  # (9, cout, cin)
    B = np.zeros((2, 9, COUT, CIN), np.float32)
    for s in (0, 1):
        suf = np.ones((COUT, CIN), np.float32)
        for k in range(8, -1, -1):
            B[s, k] = kflat[k] * suf
            if k % 2 == s:
                suf = suf * kflat[k]
    # merged conv lhsT per parity: (96, 3, 128): rows (ki,cin), cols
    # [B_raw (aeg) | Wc (conv)]; B_raw nonzero only where SIGMA[s][k]==0
    wc_k = conv_w.reshape(COUT, CIN, 9)  # (cout, cin, k)
    wm_p = np.zeros((2, 96, 3, 128), np.float32)
    for s in (0, 1):
        for ki in range(3):
            for kj in range(3):
                k = ki * 3 + kj
                blk = slice(32 * ki, 32 * ki + 32)
                wm_p[s, blk, kj, 64:128] = wc_k[:, :, k].T
                if SIGMA[s][k] == 0:
                    wm_p[s, blk, kj, 0:64] = B[s, k].T
    wm_p = wm_p.astype(bf16)
    ROW_ORDER = {
        (0, 0): [3, 0, 6], (0, 1): [1, 4, 7], (0, 2): [5, 2, 8],
        (1, 0): [6, 0, 3], (1, 1): [4, 1, 7], (1, 2): [2, 5, 8],
    }
    b_p = np.zeros((2, 96, 3, COUT), np.float32)
    for s in (0, 1):
        for kj in range(3):
            for ri, k in enumerate(ROW_ORDER[(s, kj)]):
                b_p[s, 32 * ri : 32 * ri + 32, kj] = B[s, k].T
    b_p = b_p.astype(bf16)
    bias_p = np.ascontiguousarray(conv_b.reshape(COUT, 1), np.float32)

    in_maps = []
    for core in range(N_CORES):
        n, h = divmod(core, 2)
        slab = xp[n, :, 32 * h : 32 * h + ROWS, :]  # (32, 34, 66) f32
        planes = np.zeros((4, CIN, ROWS, PLP), np.float32)
        for q in (0, 1):
            for r in range(ROWS):
                b = (q + r) % 2
                cols = slab[:, r, b::2]  # 33 columns
                planes[q, :, r, : cols.shape[1]] = cols
        planes[2, :, :, :PLP - 1] = planes[0, :, :, 1:]
        planes[3, :, :, :PLP - 1] = planes[1, :, :, 1:]
        xp_core = np.ascontiguousarray(
            planes.transpose(1, 0, 2, 3).reshape(CIN, 4 * PLSZ)
        ).astype(bf16)
        # partition-stacked row-shifted plane sets for the K=96 conv chunks:
        # xa group r holds plane (r%2) shifted down r rows (+ col-shift copy),
        # xb group r holds plane ((r+1)%2) likewise.
        xab = np.zeros((2, 3, CIN, 2, ROWS, PLP), np.float32)
        for c in (0, 1):
            for r in range(3):
                q = (c + r) % 2
                xab[c, r, :, 0, : ROWS - r] = planes[q, :, r:]
                xab[c, r, :, 1, : ROWS - r] = planes[2 + q, :, r:]
        xa_core = np.ascontiguousarray(xab[0].reshape(96, 2 * PLSZ)).astype(bf16)
        xb_core = np.ascontiguousarray(xab[1].reshape(96, 2 * PLSZ)).astype(bf16)
        in_maps.append({
            "xp": xp_core,
            "xa": xa_core,
            "xb": xb_core,
            "wm0": wm_p[0],
            "wm1": wm_p[1],
            "b0": b_p[0],
            "b1": b_p[1],
            "bias": bias_p,
        })
    return in_maps


_nc_cache = None


def kernel(x, weight, conv_w, conv_b, trace=False):
    global _nc_cache, _last_results
    x = np.asarray(x, np.float32)
    weight = np.asarray(weight, np.float32)
    conv_w = np.asarray(conv_w, np.float32)
    conv_b = np.asarray(conv_b, np.float32)

    if _nc_cache is None:
        _nc_cache = build_nc()
    nc = _nc_cache
    in_maps = _host_prep(x, weight, conv_w, conv_b)
    res = run_bass_kernel_spmd(nc, in_maps, core_ids=list(range(N_CORES)), trace=trace)
    _last_results = res

    out = np.empty((N, COUT, H, W), np.float32)
    for core in range(N_CORES):
        n, h = divmod(core, 2)
        out[n, :, 32 * h : 32 * h + 32, :] = (
            res.results[core]["out"].astype(np.float32).reshape(COUT, OH, OW)
        )
    return out
